# revision 3
# baseline (speedup 1.0000x reference)
"""APPNP (MLP + 10-step personalized-pagerank propagation) on 8 trn2 NeuronCores.

Strategy:
- Nodes are dst-sharded across 8 cores (12500 each).
- MLP (x @ W1 -> relu -> @ W2) runs on the tensor engine per core over the
  core's node shard, with x pre-transposed on host (contraction dim on
  partitions) and b1 folded in as an extra ones-row of x.
- Propagation uses the factorized GCN norm: A_hat h = dinv * ((A+I) (dinv*h)),
  so no per-edge norm values are needed: per step each core computes
  g = dinv*h on its shard, AllGathers g into a full table in DRAM, gathers
  g[src] for each in-edge of its shard via indirect DMA into a degree-uniform
  slot layout, reduces slots per dst with one vector-engine reduction per
  128-dst block, adds the self-loop term and the alpha*h0 term.
- Slot layout: per core, dsts sorted by in-degree desc; sorted position
  s <-> (block b = s//128, lane = s%128). Block b's slot count = max degree
  in block (degree-sorted => tiny padding). Pad slots gather a zero row.
"""
import numpy as np

_LAST_NC = None
_LAST_IN_MAPS = None

K = 10
ALPHA = 0.1
N_NODES = 100000
N_CORES = 8
NS = N_NODES // N_CORES          # 12500 dsts per core
NB = 98                           # ceil(12544/128) blocks (12544 = 128*98)
NRANK = 128 * NB                  # 12544 padded ranks per core
SHARD_ROWS = NRANK + 1            # +1 zero row for pad gathers
IN_CH, HID_CH, OUT_CH = 500, 64, 16
KIN = 512                         # padded in_ch (500 feats + 1 bias + pad)
P = 128


def _build_host_data(x, edge_index, W1, b1, W2, b2):
    x = np.asarray(x, dtype=np.float32)
    ei = np.asarray(edge_index)
    src = ei[0].astype(np.int64)
    dst = ei[1].astype(np.int64)

    deg = np.bincount(dst, minlength=N_NODES).astype(np.float32) + 1.0
    dinv = 1.0 / np.sqrt(deg)

    # per-core degree sort of the core's dst shard; global row map for g table
    row_of_node = np.empty(N_NODES, dtype=np.int64)
    perm_per_core = []          # natural ids in sorted order per core
    for c in range(N_CORES):
        ids = np.arange(c * NS, (c + 1) * NS)
        order = np.argsort(-deg[ids], kind="stable")
        ids_sorted = ids[order]
        perm_per_core.append(ids_sorted)
        s = np.arange(NS)
        lane = s % P
        b = s // P
        row_of_node[ids_sorted] = c * SHARD_ROWS + lane * NB + b
    zero_row_of_core0 = NRANK  # row index (within shard) that stays zero

    # per-core slot tables
    per_core = []
    dst_core = dst // NS
    for c in range(N_CORES):
        m = dst_core == c
        src_c = src[m]
        dst_c = dst[m]
        ids_sorted = perm_per_core[c]
        # sorted position of each dst in this core
        pos_of = np.empty(NS, dtype=np.int64)
        pos_of[ids_sorted - c * NS] = np.arange(NS)
        pos = pos_of[dst_c - c * NS]
        lane = pos % P
        blk = pos // P
        degs = deg[ids_sorted].astype(np.int64) - 1   # in-edges only
        d_b = np.zeros(NB, dtype=np.int64)
        for b in range(NB):
            seg = degs[b * P:(b + 1) * P]
            d_b[b] = seg.max() if len(seg) else 0
        d_b = np.maximum(d_b, 0)
        col_off = np.zeros(NB + 1, dtype=np.int64)
        col_off[1:] = np.cumsum(d_b)
        T_g = int(col_off[-1])

        # slot fill: order edges by (blk, lane) then slot rank within dst
        idx_tab = np.full((P, T_g), zero_row_of_core0, dtype=np.int64)
        order2 = np.lexsort((src_c, pos))   # group by dst pos
        pos_s = pos[order2]
        src_s = src_c[order2]
        # rank within each dst
        counts = np.bincount(pos_s, minlength=NS)
        rank = np.arange(len(pos_s)) - np.repeat(
            np.concatenate(([0], np.cumsum(counts)))[:-1], counts)
        lane_s = pos_s % P
        blk_s = pos_s // P
        cols = col_off[blk_s] + rank
        idx_tab[lane_s, cols] = row_of_node[src_s]
        per_core.append(dict(idx=idx_tab.astype(np.int32), d_b=d_b,
                             col_off=col_off, T_g=T_g,
                             ids_sorted=ids_sorted))

    # MLP host prep per core: xT [128, 4, NRANK] fp32, column order = sorted pos
    W1p = np.zeros((KIN, HID_CH), dtype=np.float32)
    W1p[:IN_CH] = np.asarray(W1, dtype=np.float32)
    W1p[IN_CH] = np.asarray(b1, dtype=np.float32)
    W1p_t = W1p.reshape(4, P, HID_CH).transpose(1, 0, 2).copy()  # [128,4,64]
    for c in range(N_CORES):
        ids_sorted = per_core[c]["ids_sorted"]
        xp = np.zeros((KIN, NRANK), dtype=np.float32)
        xp[:IN_CH, :NS] = x[ids_sorted].T
        xp[IN_CH, :NS] = 1.0
        per_core[c]["xT"] = xp.reshape(4, P, NRANK).transpose(1, 0, 2).copy()
        dv = np.zeros((P, NB), dtype=np.float32)
        s = np.arange(NS)
        dv[s % P, s // P] = dinv[ids_sorted]
        per_core[c]["dinv"] = dv
    return per_core, W1p_t, np.asarray(W2, np.float32), np.asarray(b2, np.float32)


def _build_bass(d_b_list, T_g_list, n_queues=4):
    import concourse.bacc as bacc
    import concourse.mybir as mybir
    import concourse.tile as tile
    import concourse.bass as bass

    # all cores share one program; use the max structure and per-core idx data.
    # d_b differs per core -> use per-column gather driven by a SHARED column
    # count T_max, with per-core idx tables padded to T_max (pad cols gather
    # the zero row into a scratch slot tile and reduce into a junk agg block).
    # Simpler: use the same d_b schedule for all cores = elementwise max over
    # cores (computed on host, passed in d_b_list as the shared schedule).
    d_b = d_b_list
    T_g = int(np.sum(d_b))
    DMAX = int(max(d_b)) if len(d_b) else 1

    nc = bacc.Bacc(None, num_devices=N_CORES, num_swdge_queues=n_queues)
    xT = nc.dram_tensor("xT", [P, 4, NRANK], mybir.dt.float32, kind="ExternalInput")
    W1p = nc.dram_tensor("W1p", [P, 4, HID_CH], mybir.dt.float32, kind="ExternalInput")
    W2 = nc.dram_tensor("W2", [HID_CH, OUT_CH], mybir.dt.float32, kind="ExternalInput")
    b2 = nc.dram_tensor("b2", [OUT_CH, 1], mybir.dt.float32, kind="ExternalInput")
    dinv_in = nc.dram_tensor("dinv", [P, NB], mybir.dt.float32, kind="ExternalInput")
    idx_in = nc.dram_tensor("idx", [P, max(T_g, 1)], mybir.dt.int32, kind="ExternalInput")
    h_out = nc.dram_tensor("h_out", [P, NB * OUT_CH], mybir.dt.float32, kind="ExternalOutput")

    gshard = nc.dram_tensor("gshard", [SHARD_ROWS, OUT_CH], mybir.dt.float32)
    Gtab = nc.dram_tensor("Gtab", [SHARD_ROWS * N_CORES, OUT_CH], mybir.dt.float32)

    dt = mybir.dt.float32
    with tile.TileContext(nc) as tc:
        with tc.tile_pool(name="persist", bufs=1) as pers, \
             tc.tile_pool(name="mlp", bufs=3) as mpool, \
             tc.tile_pool(name="slot", bufs=3) as spool, \
             tc.tile_pool(name="ps", bufs=2, space="PSUM") as pp, \
             tc.tile_pool(name="pst", bufs=2, space="PSUM") as ppt:

            # persistent tiles
            idx_t = pers.tile([P, max(T_g, 1)], mybir.dt.int32)
            nc.gpsimd.dma_start(idx_t[:], idx_in[:])
            dinv_t = pers.tile([P, NB], dt)
            nc.gpsimd.dma_start(dinv_t[:], dinv_in[:])
            w1_t = pers.tile([P, 4, HID_CH], dt)
            nc.gpsimd.dma_start(w1_t[:], W1p[:])
            w2_t = pers.tile([HID_CH, OUT_CH], dt)
            nc.gpsimd.dma_start(w2_t[:], W2[:])
            b2_t = pers.tile([OUT_CH, 1], dt)
            nc.gpsimd.dma_start(b2_t[:], b2[:])
            ident = pers.tile([P, P], dt)
            from concourse.masks import make_identity
            make_identity(nc, ident[:])

            h0s_t = pers.tile([P, NB, OUT_CH], dt)   # alpha * h0
            h_t = pers.tile([P, NB, OUT_CH], dt)     # current h
            g_t = pers.tile([P, NB, OUT_CH], dt)     # dinv * h
            agg_t = pers.tile([P, NB, OUT_CH], dt)
            zrow = pers.tile([1, OUT_CH], dt)
            nc.gpsimd.memset(zrow[:], 0.0)
            nc.gpsimd.dma_start(gshard.ap()[NRANK:NRANK + 1, :], zrow[:])

            # ---- MLP ----
            NT = NRANK // KIN  # 24.5 -> handle in 2 tile sizes; NRANK=12544=24*512+256
            tiles = [(t * KIN, KIN) for t in range(NRANK // KIN)]
            rem = NRANK - (NRANK // KIN) * KIN
            if rem:
                tiles.append(((NRANK // KIN) * KIN, rem))
            for (c0, w) in tiles:
                xt = mpool.tile([P, 4, KIN], dt, tag="xt")
                nc.gpsimd.dma_start(xt[:, :, :w], xT[:, :, c0:c0 + w])
                ps1 = pp.tile([HID_CH, KIN], dt, tag="ps1")
                for k in range(4):
                    nc.tensor.matmul(ps1[:, :w], w1_t[:, k, :], xt[:, k, :w],
                                     start=(k == 0), stop=(k == 3))
                h1 = mpool.tile([HID_CH, KIN], dt, tag="h1")
                nc.vector.tensor_scalar_max(h1[:, :w], ps1[:, :w], 0.0)
                ps2 = pp.tile([OUT_CH, KIN], dt, tag="ps2")
                nc.tensor.matmul(ps2[:, :w], w2_t[:], h1[:, :w],
                                 start=True, stop=True)
                hT = mpool.tile([OUT_CH, KIN], dt, tag="hT")
                nc.vector.tensor_tensor(hT[:, :w], ps2[:, :w],
                                        b2_t[:].to_broadcast([OUT_CH, w]),
                                        op=mybir.AluOpType.add)
                for j in range(w // P):
                    b = (c0 + j * P) // P
                    pst = ppt.tile([P, OUT_CH], dt, tag="pst")
                    nc.tensor.transpose(pst[:], hT[:, j * P:(j + 1) * P],
                                        ident[:OUT_CH, :OUT_CH])
                    nc.vector.tensor_copy(h0s_t[:, b, :], pst[:])
            # h = h0 ; h0s = alpha*h0
            nc.vector.tensor_copy(h_t[:], h0s_t[:])
            nc.vector.tensor_scalar_mul(h0s_t[:], h0s_t[:], ALPHA)

            # ---- propagation steps ----
            col_off = np.zeros(NB + 1, dtype=np.int64)
            col_off[1:] = np.cumsum(d_b)

            def step_body(_i):
                import concourse.bass as bass_
                # g = dinv * h
                nc.vector.tensor_tensor(
                    g_t[:], h_t[:],
                    dinv_t[:].rearrange("p (b o) -> p b o", o=1).to_broadcast([P, NB, OUT_CH]),
                    op=mybir.AluOpType.mult)
                nc.gpsimd.dma_start(gshard.ap()[:NRANK, :], g_t[:])
                nc.gpsimd.collective_compute(
                    "AllGather", mybir.AluOpType.bypass,
                    replica_groups=[list(range(N_CORES))],
                    ins=[gshard.ap()[:, :]],
                    outs=[Gtab.ap()[:, :]],
                )
                qi = 0
                for b in range(NB):
                    db = int(d_b[b])
                    if db == 0:
                        nc.gpsimd.memset(agg_t[:, b, :], 0.0)
                        continue
                    st = spool.tile([P, DMAX, OUT_CH], dt, tag="slot")
                    for s in range(db):
                        col = int(col_off[b] + s)
                        inst = nc.gpsimd.indirect_dma_start(
                            out=st[:, s, :], out_offset=None, in_=Gtab[:],
                            in_offset=bass_.IndirectOffsetOnAxis(
                                ap=idx_t[:, col:col + 1], axis=0))
                        q = qi % n_queues
                        qi += 1
                        if q:
                            inst.ins.queue = f"qPoolDynamic{q}"
                    nc.vector.reduce_sum(
                        agg_t[:, b, :],
                        st[:, :db, :].rearrange("p s c -> p c s"),
                        axis=mybir.AxisListType.X)
                # h = 0.9 * dinv * (agg + g) + alpha*h0
                nc.vector.tensor_add(agg_t[:], agg_t[:], g_t[:])
                nc.vector.tensor_tensor(
                    agg_t[:], agg_t[:],
                    dinv_t[:].rearrange("p (b o) -> p b o", o=1).to_broadcast([P, NB, OUT_CH]),
                    op=mybir.AluOpType.mult)
                nc.vector.tensor_scalar_mul(agg_t[:], agg_t[:], 1.0 - ALPHA)
                nc.vector.tensor_add(h_t[:], agg_t[:], h0s_t[:])

            for _step in range(K):
                step_body(_step)

            nc.gpsimd.dma_start(h_out[:], h_t[:])
    nc.compile()
    return nc


def kernel(x, edge_index, W1, b1, W2, b2):
    per_core, W1p_t, W2a, b2a = _build_host_data(x, edge_index, W1, b1, W2, b2)

    # shared gather schedule: elementwise max of d_b across cores
    d_b = np.max(np.stack([pc["d_b"] for pc in per_core]), axis=0)
    T_g = int(d_b.sum())
    col_off = np.zeros(NB + 1, dtype=np.int64)
    col_off[1:] = np.cumsum(d_b)

    in_maps = []
    for c in range(N_CORES):
        pc = per_core[c]
        idx_pad = np.full((P, T_g), NRANK, dtype=np.int32)  # zero row of core 0
        for b in range(NB):
            db_c = int(pc["d_b"][b])
            if db_c:
                idx_pad[:, col_off[b]:col_off[b] + db_c] = \
                    pc["idx"][:, pc["col_off"][b]:pc["col_off"][b] + db_c]
        in_maps.append({
            "xT": pc["xT"],
            "W1p": W1p_t,
            "W2": W2a,
            "b2": b2a.reshape(OUT_CH, 1),
            "dinv": pc["dinv"],
            "idx": idx_pad,
        })

    nc = _build_bass(d_b, [T_g] * N_CORES)
    global _LAST_NC, _LAST_IN_MAPS
    _LAST_NC, _LAST_IN_MAPS = nc, in_maps
    from concourse import bass_utils
    res = bass_utils.run_bass_kernel_spmd(nc, in_maps, core_ids=list(range(N_CORES)))

    out = np.zeros((N_NODES, OUT_CH), dtype=np.float32)
    for c in range(N_CORES):
        hc = res.results[c]["h_out"].reshape(P, NB, OUT_CH)
        ids_sorted = per_core[c]["ids_sorted"]
        s = np.arange(NS)
        out[ids_sorted] = hc[s % P, s // P, :]
    return out


# revision 7
# speedup vs baseline: 1.3336x; 1.3336x over previous
"""APPNP (MLP + 10-step personalized-pagerank propagation) on 8 trn2 NeuronCores.

Strategy:
- Nodes are dst-sharded across 8 cores (12500 each).
- MLP (x @ W1 -> relu -> @ W2) runs on the tensor engine per core over the
  core's node shard, with x pre-transposed on host (contraction dim on
  partitions) and b1 folded in as an extra ones-row of x.
- Propagation uses the factorized GCN norm: A_hat h = dinv * ((A+I) (dinv*h)),
  so no per-edge norm values are needed: per step each core computes
  g = dinv*h on its shard, AllGathers g into a full table in DRAM, gathers
  g[src] for each in-edge of its shard via indirect DMA into a degree-uniform
  slot layout, reduces slots per dst with one vector-engine reduction per
  128-dst block, adds the self-loop term and the alpha*h0 term.
- Slot layout: per core, dsts sorted by in-degree desc; sorted position
  s <-> (block b = s//128, lane = s%128). Block b's slot count = max degree
  in block (degree-sorted => tiny padding). Pad slots gather a zero row.
"""
import numpy as np

_LAST_NC = None
_LAST_IN_MAPS = None

K = 10
ALPHA = 0.1
N_NODES = 100000
N_CORES = 8
NS = N_NODES // N_CORES          # 12500 dsts per core
NB = 98                           # ceil(12544/128) blocks (12544 = 128*98)
NRANK = 128 * NB                  # 12544 padded ranks per core
SHARD_ROWS = NRANK + 1            # +1 zero row for pad gathers
IN_CH, HID_CH, OUT_CH = 500, 64, 16
KIN = 512                         # padded in_ch (500 feats + 1 bias + pad)
P = 128



def _make_groups(d_b, sgc=288):
    groups = []   # (b0, nb, dmax_g)
    b = 0
    while b < NB:
        dmax_g = max(int(d_b[b]), 1)
        nb = 1
        while (b + nb < NB and (nb + 1) * dmax_g <= sgc
               and int(d_b[b + nb]) <= dmax_g):
            nb += 1
        groups.append((b, nb, dmax_g))
        b += nb
    return groups


def _build_host_data(x, edge_index, W1, b1, W2, b2):
    x = np.asarray(x, dtype=np.float32)
    ei = np.asarray(edge_index)
    src = ei[0].astype(np.int64)
    dst = ei[1].astype(np.int64)

    deg = np.bincount(dst, minlength=N_NODES).astype(np.float32) + 1.0
    dinv = 1.0 / np.sqrt(deg)

    # per-core degree sort of the core's dst shard; global row map for g table
    row_of_node = np.empty(N_NODES, dtype=np.int64)
    perm_per_core = []          # natural ids in sorted order per core
    for c in range(N_CORES):
        ids = np.arange(c * NS, (c + 1) * NS)
        order = np.argsort(-deg[ids], kind="stable")
        ids_sorted = ids[order]
        perm_per_core.append(ids_sorted)
        s = np.arange(NS)
        lane = s % P
        b = s // P
        row_of_node[ids_sorted] = c * SHARD_ROWS + lane * NB + b
    zero_row_of_core0 = NRANK  # row index (within shard) that stays zero

    # per-core slot tables
    per_core = []
    dst_core = dst // NS
    for c in range(N_CORES):
        m = dst_core == c
        src_c = src[m]
        dst_c = dst[m]
        ids_sorted = perm_per_core[c]
        # sorted position of each dst in this core
        pos_of = np.empty(NS, dtype=np.int64)
        pos_of[ids_sorted - c * NS] = np.arange(NS)
        pos = pos_of[dst_c - c * NS]
        lane = pos % P
        blk = pos // P
        degs = deg[ids_sorted].astype(np.int64) - 1   # in-edges only
        d_b = np.zeros(NB, dtype=np.int64)
        for b in range(NB):
            seg = degs[b * P:(b + 1) * P]
            d_b[b] = seg.max() if len(seg) else 0
        d_b = np.maximum(d_b, 0)
        col_off = np.zeros(NB + 1, dtype=np.int64)
        col_off[1:] = np.cumsum(d_b)
        T_g = int(col_off[-1])

        # slot fill: order edges by (blk, lane) then slot rank within dst
        idx_tab = np.full((P, T_g), zero_row_of_core0, dtype=np.int64)
        order2 = np.lexsort((src_c, pos))   # group by dst pos
        pos_s = pos[order2]
        src_s = src_c[order2]
        # rank within each dst
        counts = np.bincount(pos_s, minlength=NS)
        rank = np.arange(len(pos_s)) - np.repeat(
            np.concatenate(([0], np.cumsum(counts)))[:-1], counts)
        lane_s = pos_s % P
        blk_s = pos_s // P
        cols = col_off[blk_s] + rank
        idx_tab[lane_s, cols] = row_of_node[src_s]
        per_core.append(dict(idx=idx_tab.astype(np.int32), d_b=d_b,
                             col_off=col_off, T_g=T_g,
                             ids_sorted=ids_sorted))

    # MLP host prep per core: xT [128, 4, NRANK] fp32, column order = sorted pos
    W1p = np.zeros((KIN, HID_CH), dtype=np.float32)
    W1p[:IN_CH] = np.asarray(W1, dtype=np.float32)
    W1p[IN_CH] = np.asarray(b1, dtype=np.float32)
    W1p_t = W1p.reshape(4, P, HID_CH).transpose(1, 0, 2).copy()  # [128,4,64]
    for c in range(N_CORES):
        ids_sorted = per_core[c]["ids_sorted"]
        xp = np.zeros((KIN, NRANK), dtype=np.float32)
        xp[:IN_CH, :NS] = x[ids_sorted].T
        xp[IN_CH, :NS] = 1.0
        per_core[c]["xT"] = xp.reshape(4, P, NRANK).transpose(1, 0, 2).copy()
        dv = np.zeros((P, NB), dtype=np.float32)
        s = np.arange(NS)
        dv[s % P, s // P] = dinv[ids_sorted]
        per_core[c]["dinv"] = dv
    return per_core, W1p_t, np.asarray(W2, np.float32), np.asarray(b2, np.float32)


def _build_bass(d_b_list, T_g_list, n_queues=4, k_steps=K, do_gather=True, do_allgather=True, slot_bufs=4):
    import concourse.bacc as bacc
    import concourse.mybir as mybir
    import concourse.tile as tile
    import concourse.bass as bass

    # all cores share one program; use the max structure and per-core idx data.
    # d_b differs per core -> use per-column gather driven by a SHARED column
    # count T_max, with per-core idx tables padded to T_max (pad cols gather
    # the zero row into a scratch slot tile and reduce into a junk agg block).
    # Simpler: use the same d_b schedule for all cores = elementwise max over
    # cores (computed on host, passed in d_b_list as the shared schedule).
    d_b = d_b_list
    _groups = _make_groups(d_b)
    T_g = int(sum(nb * dm for (_b0, nb, dm) in _groups))
    DMAX = int(max(d_b)) if len(d_b) else 1

    nc = bacc.Bacc(None, num_devices=N_CORES, num_swdge_queues=n_queues,
                   dynamic_dma_scratch_size=65536)
    xT = nc.dram_tensor("xT", [P, 4, NRANK], mybir.dt.float32, kind="ExternalInput")
    W1p = nc.dram_tensor("W1p", [P, 4, HID_CH], mybir.dt.float32, kind="ExternalInput")
    W2 = nc.dram_tensor("W2", [HID_CH, OUT_CH], mybir.dt.float32, kind="ExternalInput")
    b2 = nc.dram_tensor("b2", [OUT_CH, 1], mybir.dt.float32, kind="ExternalInput")
    dinv_in = nc.dram_tensor("dinv", [P, NB], mybir.dt.float32, kind="ExternalInput")
    idx_in = nc.dram_tensor("idx", [P, max(T_g, 1)], mybir.dt.int32, kind="ExternalInput")
    h_out = nc.dram_tensor("h_out", [P, NB * OUT_CH], mybir.dt.float32, kind="ExternalOutput")

    gshard = nc.dram_tensor("gshard", [SHARD_ROWS, OUT_CH], mybir.dt.float32)
    Gtab = nc.dram_tensor("Gtab", [SHARD_ROWS * N_CORES, OUT_CH], mybir.dt.float32)

    dt = mybir.dt.float32
    with tile.TileContext(nc) as tc:
        with tc.tile_pool(name="persist", bufs=1) as pers, \
             tc.tile_pool(name="mlp", bufs=3) as mpool, \
             tc.tile_pool(name="slot", bufs=slot_bufs) as spool, \
             tc.tile_pool(name="ps", bufs=2, space="PSUM") as pp, \
             tc.tile_pool(name="pst", bufs=2, space="PSUM") as ppt:

            # persistent tiles
            idx_t = pers.tile([P, max(T_g, 1)], mybir.dt.int32)
            nc.gpsimd.dma_start(idx_t[:], idx_in[:])
            dinv_t = pers.tile([P, NB], dt)
            nc.gpsimd.dma_start(dinv_t[:], dinv_in[:])
            w1_t = pers.tile([P, 4, HID_CH], dt)
            nc.gpsimd.dma_start(w1_t[:], W1p[:])
            w2_t = pers.tile([HID_CH, OUT_CH], dt)
            nc.gpsimd.dma_start(w2_t[:], W2[:])
            b2_t = pers.tile([OUT_CH, 1], dt)
            nc.gpsimd.dma_start(b2_t[:], b2[:])
            ident = pers.tile([P, P], dt)
            from concourse.masks import make_identity
            make_identity(nc, ident[:])

            h0s_t = pers.tile([P, NB, OUT_CH], dt)   # alpha * h0
            h_t = pers.tile([P, NB, OUT_CH], dt)     # current h
            g_t = pers.tile([P, NB, OUT_CH], dt)     # dinv * h
            agg_t = pers.tile([P, NB, OUT_CH], dt)
            zrow = pers.tile([1, OUT_CH], dt)
            nc.gpsimd.memset(zrow[:], 0.0)
            nc.gpsimd.dma_start(gshard.ap()[NRANK:NRANK + 1, :], zrow[:])

            # ---- MLP ----
            NT = NRANK // KIN  # 24.5 -> handle in 2 tile sizes; NRANK=12544=24*512+256
            tiles = [(t * KIN, KIN) for t in range(NRANK // KIN)]
            rem = NRANK - (NRANK // KIN) * KIN
            if rem:
                tiles.append(((NRANK // KIN) * KIN, rem))
            for (c0, w) in tiles:
                xt = mpool.tile([P, 4, KIN], dt, tag="xt")
                nc.gpsimd.dma_start(xt[:, :, :w], xT[:, :, c0:c0 + w])
                ps1 = pp.tile([HID_CH, KIN], dt, tag="ps1")
                for k in range(4):
                    nc.tensor.matmul(ps1[:, :w], w1_t[:, k, :], xt[:, k, :w],
                                     start=(k == 0), stop=(k == 3))
                h1 = mpool.tile([HID_CH, KIN], dt, tag="h1")
                nc.vector.tensor_scalar_max(h1[:, :w], ps1[:, :w], 0.0)
                ps2 = pp.tile([OUT_CH, KIN], dt, tag="ps2")
                nc.tensor.matmul(ps2[:, :w], w2_t[:], h1[:, :w],
                                 start=True, stop=True)
                hT = mpool.tile([OUT_CH, KIN], dt, tag="hT")
                nc.vector.tensor_tensor(hT[:, :w], ps2[:, :w],
                                        b2_t[:].to_broadcast([OUT_CH, w]),
                                        op=mybir.AluOpType.add)
                for j in range(w // P):
                    b = (c0 + j * P) // P
                    pst = ppt.tile([P, OUT_CH], dt, tag="pst")
                    nc.tensor.transpose(pst[:], hT[:, j * P:(j + 1) * P],
                                        ident[:OUT_CH, :OUT_CH])
                    nc.vector.tensor_copy(h0s_t[:, b, :], pst[:])
            # h = h0 ; h0s = alpha*h0
            nc.vector.tensor_copy(h_t[:], h0s_t[:])
            nc.vector.tensor_scalar_mul(h0s_t[:], h0s_t[:], ALPHA)

            # ---- propagation steps ----
            SGC = 288
            groups = _make_groups(d_b, SGC)
            # uniform schedule: every block in a group has dmax_g columns
            d_u = np.zeros(NB, dtype=np.int64)
            for (b0, nb, dmax_g) in groups:
                d_u[b0:b0 + nb] = dmax_g
            col_off = np.zeros(NB + 1, dtype=np.int64)
            col_off[1:] = np.cumsum(d_u)

            def step_body(_i):
                import concourse.bass as bass_
                # g = dinv * h
                nc.vector.tensor_tensor(
                    g_t[:], h_t[:],
                    dinv_t[:].rearrange("p (b o) -> p b o", o=1).to_broadcast([P, NB, OUT_CH]),
                    op=mybir.AluOpType.mult)
                nc.gpsimd.dma_start(gshard.ap()[:NRANK, :], g_t[:])
                if do_allgather:
                    nc.gpsimd.collective_compute(
                        "AllGather", mybir.AluOpType.bypass,
                        replica_groups=[list(range(N_CORES))],
                        ins=[gshard.ap()[:, :]],
                        outs=[Gtab.ap()[:, :]],
                    )
                qi = 0
                for (b0, nb, dmax_g) in groups:
                    if not do_gather:
                        break
                    st = spool.tile([P, SGC, OUT_CH], dt, tag="slot")
                    stv = st[:, :nb * dmax_g, :].rearrange(
                        "p (b s) c -> p b s c", b=nb)
                    for j in range(nb):
                        for s in range(dmax_g):
                            col = int(col_off[b0 + j] + s)
                            inst = nc.gpsimd.indirect_dma_start(
                                out=stv[:, j, s, :], out_offset=None, in_=Gtab[:],
                                in_offset=bass_.IndirectOffsetOnAxis(
                                    ap=idx_t[:, col:col + 1], axis=0))
                            q = qi % n_queues
                            qi += 1
                            if q:
                                inst.ins.queue = f"qPoolDynamic{q}"
                    nc.vector.reduce_sum(
                        agg_t[:, b0:b0 + nb, :].rearrange("p b c -> p c b"),
                        stv[:].rearrange("p b s c -> p c b s"),
                        axis=mybir.AxisListType.X)
                # h = 0.9 * dinv * (agg + g) + alpha*h0
                nc.vector.tensor_add(agg_t[:], agg_t[:], g_t[:])
                nc.vector.tensor_tensor(
                    agg_t[:], agg_t[:],
                    dinv_t[:].rearrange("p (b o) -> p b o", o=1).to_broadcast([P, NB, OUT_CH]),
                    op=mybir.AluOpType.mult)
                nc.vector.tensor_scalar_mul(agg_t[:], agg_t[:], 1.0 - ALPHA)
                nc.vector.tensor_add(h_t[:], agg_t[:], h0s_t[:])

            for _step in range(k_steps):
                step_body(_step)

            nc.gpsimd.dma_start(h_out[:], h_t[:])
    nc.compile()
    return nc


def kernel(x, edge_index, W1, b1, W2, b2):
    per_core, W1p_t, W2a, b2a = _build_host_data(x, edge_index, W1, b1, W2, b2)

    # shared gather schedule: elementwise max of d_b across cores
    d_b = np.max(np.stack([pc["d_b"] for pc in per_core]), axis=0)
    groups = _make_groups(d_b)
    d_u = np.zeros(NB, dtype=np.int64)
    for (b0, nb, dmax_g) in groups:
        d_u[b0:b0 + nb] = dmax_g
    T_g = int(d_u.sum())
    col_off = np.zeros(NB + 1, dtype=np.int64)
    col_off[1:] = np.cumsum(d_u)

    in_maps = []
    for c in range(N_CORES):
        pc = per_core[c]
        idx_pad = np.full((P, T_g), NRANK, dtype=np.int32)  # zero row of core 0
        for b in range(NB):
            db_c = int(pc["d_b"][b])
            if db_c:
                idx_pad[:, col_off[b]:col_off[b] + db_c] = \
                    pc["idx"][:, pc["col_off"][b]:pc["col_off"][b] + db_c]
        in_maps.append({
            "xT": pc["xT"],
            "W1p": W1p_t,
            "W2": W2a,
            "b2": b2a.reshape(OUT_CH, 1),
            "dinv": pc["dinv"],
            "idx": idx_pad,
        })

    nc = _build_bass(d_b, [T_g] * N_CORES)
    global _LAST_NC, _LAST_IN_MAPS
    _LAST_NC, _LAST_IN_MAPS = nc, in_maps
    from concourse import bass_utils
    res = bass_utils.run_bass_kernel_spmd(nc, in_maps, core_ids=list(range(N_CORES)))

    out = np.zeros((N_NODES, OUT_CH), dtype=np.float32)
    for c in range(N_CORES):
        hc = res.results[c]["h_out"].reshape(P, NB, OUT_CH)
        ids_sorted = per_core[c]["ids_sorted"]
        s = np.arange(NS)
        out[ids_sorted] = hc[s % P, s // P, :]
    return out


# revision 11
# speedup vs baseline: 47.9419x; 35.9492x over previous
"""APPNP (MLP + 10-step personalized-pagerank propagation) on 8 trn2 NeuronCores.

Strategy:
- Nodes are dst-sharded across 8 cores (12500 each).
- MLP (x @ W1 -> relu -> @ W2) runs on the tensor engine per core over the
  core's node shard, with x pre-transposed on host (contraction dim on
  partitions) and b1 folded in as an extra ones-row of x.
- Propagation uses the factorized GCN norm: A_hat h = dinv * ((A+I) (dinv*h)),
  so no per-edge norm values are needed: per step each core computes
  g = dinv*h on its shard, AllGathers g into a full table in DRAM, gathers
  g[src] for each in-edge of its shard via indirect DMA into a degree-uniform
  slot layout, reduces slots per dst with one vector-engine reduction per
  128-dst block, adds the self-loop term and the alpha*h0 term.
- Slot layout: per core, dsts sorted by in-degree desc; sorted position
  s <-> (block b = s//128, lane = s%128). Block b's slot count = max degree
  in block (degree-sorted => tiny padding). Pad slots gather a zero row.
"""
import numpy as np

_LAST_NC = None
_LAST_IN_MAPS = None

K = 10
ALPHA = 0.1
N_NODES = 100000
N_CORES = 8
NS = N_NODES // N_CORES          # 12500 dsts per core
NB = 98                           # ceil(12544/128) blocks (12544 = 128*98)
NRANK = 128 * NB                  # 12544 padded ranks per core
SHARD_ROWS = NRANK + 1            # +1 zero row for pad gathers
IN_CH, HID_CH, OUT_CH = 500, 64, 16
KIN = 512                         # padded in_ch (500 feats + 1 bias + pad)
P = 128



def _make_groups(d_b, sgc=512):
    groups = []   # (b0, nb, dmax_g)
    b = 0
    while b < NB:
        dmax_g = max(int(d_b[b]), 1)
        nb = 1
        while (b + nb < NB and (nb + 1) * dmax_g <= sgc
               and int(d_b[b + nb]) <= dmax_g):
            nb += 1
        groups.append((b, nb, dmax_g))
        b += nb
    return groups


def _build_host_data(x, edge_index, W1, b1, W2, b2):
    x = np.asarray(x, dtype=np.float32)
    ei = np.asarray(edge_index)
    src = ei[0].astype(np.int64)
    dst = ei[1].astype(np.int64)

    deg = np.bincount(dst, minlength=N_NODES).astype(np.float32) + 1.0
    dinv = 1.0 / np.sqrt(deg)

    # per-core degree sort of the core's dst shard; global row map for g table
    row_of_node = np.empty(N_NODES, dtype=np.int64)
    perm_per_core = []          # natural ids in sorted order per core
    for c in range(N_CORES):
        ids = np.arange(c * NS, (c + 1) * NS)
        order = np.argsort(-deg[ids], kind="stable")
        ids_sorted = ids[order]
        perm_per_core.append(ids_sorted)
        s = np.arange(NS)
        lane = s % P
        b = s // P
        row_of_node[ids_sorted] = c * SHARD_ROWS + lane * NB + b
    zero_row_of_core0 = NRANK  # row index (within shard) that stays zero

    # per-core slot tables
    per_core = []
    dst_core = dst // NS
    for c in range(N_CORES):
        m = dst_core == c
        src_c = src[m]
        dst_c = dst[m]
        ids_sorted = perm_per_core[c]
        # sorted position of each dst in this core
        pos_of = np.empty(NS, dtype=np.int64)
        pos_of[ids_sorted - c * NS] = np.arange(NS)
        pos = pos_of[dst_c - c * NS]
        lane = pos % P
        blk = pos // P
        degs = deg[ids_sorted].astype(np.int64) - 1   # in-edges only
        d_b = np.zeros(NB, dtype=np.int64)
        for b in range(NB):
            seg = degs[b * P:(b + 1) * P]
            d_b[b] = seg.max() if len(seg) else 0
        d_b = np.maximum(d_b, 0)
        col_off = np.zeros(NB + 1, dtype=np.int64)
        col_off[1:] = np.cumsum(d_b)
        T_g = int(col_off[-1])

        # slot fill: order edges by (blk, lane) then slot rank within dst
        idx_tab = np.full((P, T_g), zero_row_of_core0, dtype=np.int64)
        order2 = np.lexsort((src_c, pos))   # group by dst pos
        pos_s = pos[order2]
        src_s = src_c[order2]
        # rank within each dst
        counts = np.bincount(pos_s, minlength=NS)
        rank = np.arange(len(pos_s)) - np.repeat(
            np.concatenate(([0], np.cumsum(counts)))[:-1], counts)
        lane_s = pos_s % P
        blk_s = pos_s // P
        cols = col_off[blk_s] + rank
        idx_tab[lane_s, cols] = row_of_node[src_s]
        per_core.append(dict(idx=idx_tab.astype(np.int32), d_b=d_b,
                             col_off=col_off, T_g=T_g,
                             ids_sorted=ids_sorted))

    # MLP host prep per core: xT [128, 4, NRANK] fp32, column order = sorted pos
    W1p = np.zeros((KIN, HID_CH), dtype=np.float32)
    W1p[:IN_CH] = np.asarray(W1, dtype=np.float32)
    W1p[IN_CH] = np.asarray(b1, dtype=np.float32)
    W1p_t = W1p.reshape(4, P, HID_CH).transpose(1, 0, 2).copy()  # [128,4,64]
    for c in range(N_CORES):
        ids_sorted = per_core[c]["ids_sorted"]
        xp = np.zeros((KIN, NRANK), dtype=np.float32)
        xp[:IN_CH, :NS] = x[ids_sorted].T
        xp[IN_CH, :NS] = 1.0
        per_core[c]["xT"] = xp.reshape(4, P, NRANK).transpose(1, 0, 2).copy()
        dv = np.zeros((P, NB), dtype=np.float32)
        s = np.arange(NS)
        dv[s % P, s // P] = dinv[ids_sorted]
        per_core[c]["dinv"] = dv
    return per_core, W1p_t, np.asarray(W2, np.float32), np.asarray(b2, np.float32)


def _build_bass(d_b_list, T_g_list, n_queues=4, k_steps=K, do_gather=True, do_allgather=True, slot_bufs=3):
    import concourse.bacc as bacc
    import concourse.mybir as mybir
    import concourse.tile as tile
    import concourse.bass as bass

    # all cores share one program; use the max structure and per-core idx data.
    # d_b differs per core -> use per-column gather driven by a SHARED column
    # count T_max, with per-core idx tables padded to T_max (pad cols gather
    # the zero row into a scratch slot tile and reduce into a junk agg block).
    # Simpler: use the same d_b schedule for all cores = elementwise max over
    # cores (computed on host, passed in d_b_list as the shared schedule).
    d_b = d_b_list
    _groups = _make_groups(d_b)
    T_g = int(sum(nb * dm for (_b0, nb, dm) in _groups))
    DMAX = int(max(d_b)) if len(d_b) else 1

    nc = bacc.Bacc(None, num_devices=N_CORES, num_swdge_queues=n_queues,
                   dynamic_dma_scratch_size=65536)
    xT = nc.dram_tensor("xT", [P, 4, NRANK], mybir.dt.float32, kind="ExternalInput")
    W1p = nc.dram_tensor("W1p", [P, 4, HID_CH], mybir.dt.float32, kind="ExternalInput")
    W2 = nc.dram_tensor("W2", [HID_CH, OUT_CH], mybir.dt.float32, kind="ExternalInput")
    b2 = nc.dram_tensor("b2", [OUT_CH, 1], mybir.dt.float32, kind="ExternalInput")
    dinv_in = nc.dram_tensor("dinv", [P, NB], mybir.dt.float32, kind="ExternalInput")
    idx_in = nc.dram_tensor("idx", [P, max(T_g, 1)], mybir.dt.int32, kind="ExternalInput")
    h_out = nc.dram_tensor("h_out", [P, NB * OUT_CH], mybir.dt.float32, kind="ExternalOutput")

    gshard = nc.dram_tensor("gshard", [SHARD_ROWS, OUT_CH], mybir.dt.float32)
    Gtab = nc.dram_tensor("Gtab", [SHARD_ROWS * N_CORES, OUT_CH], mybir.dt.float32)

    dt = mybir.dt.float32
    with tile.TileContext(nc) as tc:
        with tc.tile_pool(name="persist", bufs=1) as pers, \
             tc.tile_pool(name="ps", bufs=2, space="PSUM") as pp, \
             tc.tile_pool(name="pst", bufs=2, space="PSUM") as ppt:

            # persistent tiles
            idx_t = pers.tile([P, max(T_g, 1)], mybir.dt.int32)
            nc.gpsimd.dma_start(idx_t[:], idx_in[:])
            dinv_t = pers.tile([P, NB], dt)
            nc.gpsimd.dma_start(dinv_t[:], dinv_in[:])
            w1_t = pers.tile([P, 4, HID_CH], dt)
            nc.gpsimd.dma_start(w1_t[:], W1p[:])
            w2_t = pers.tile([HID_CH, OUT_CH], dt)
            nc.gpsimd.dma_start(w2_t[:], W2[:])
            b2_t = pers.tile([OUT_CH, 1], dt)
            nc.gpsimd.dma_start(b2_t[:], b2[:])
            ident = pers.tile([P, P], dt)
            from concourse.masks import make_identity
            make_identity(nc, ident[:])

            h0s_t = pers.tile([P, NB, OUT_CH], dt)   # alpha * h0
            h_t = pers.tile([P, NB, OUT_CH], dt)     # current h
            g_t = pers.tile([P, NB, OUT_CH], dt)     # dinv * h
            agg_t = pers.tile([P, NB, OUT_CH], dt)
            zrow = pers.tile([1, OUT_CH], dt)
            nc.gpsimd.memset(zrow[:], 0.0)
            nc.gpsimd.dma_start(gshard.ap()[NRANK:NRANK + 1, :], zrow[:])

            # ---- MLP ----
            mlp_scope = tc.tile_pool(name="mlp", bufs=3)
            mpool = mlp_scope.__enter__()
            tiles = [(t * KIN, KIN) for t in range(NRANK // KIN)]
            rem = NRANK - (NRANK // KIN) * KIN
            if rem:
                tiles.append(((NRANK // KIN) * KIN, rem))
            for (c0, w) in tiles:
                xt = mpool.tile([P, 4, KIN], dt, tag="xt")
                nc.gpsimd.dma_start(xt[:, :, :w], xT[:, :, c0:c0 + w])
                ps1 = pp.tile([HID_CH, KIN], dt, tag="ps1")
                for k in range(4):
                    nc.tensor.matmul(ps1[:, :w], w1_t[:, k, :], xt[:, k, :w],
                                     start=(k == 0), stop=(k == 3))
                h1 = mpool.tile([HID_CH, KIN], dt, tag="h1")
                nc.vector.tensor_scalar_max(h1[:, :w], ps1[:, :w], 0.0)
                ps2 = pp.tile([OUT_CH, KIN], dt, tag="ps2")
                nc.tensor.matmul(ps2[:, :w], w2_t[:], h1[:, :w],
                                 start=True, stop=True)
                hT = mpool.tile([OUT_CH, KIN], dt, tag="hT")
                nc.vector.tensor_tensor(hT[:, :w], ps2[:, :w],
                                        b2_t[:].to_broadcast([OUT_CH, w]),
                                        op=mybir.AluOpType.add)
                for j in range(w // P):
                    b = (c0 + j * P) // P
                    pst = ppt.tile([P, OUT_CH], dt, tag="pst")
                    nc.tensor.transpose(pst[:], hT[:, j * P:(j + 1) * P],
                                        ident[:OUT_CH, :OUT_CH])
                    nc.vector.tensor_copy(h0s_t[:, b, :], pst[:])
            # h = h0 ; h0s = alpha*h0
            nc.vector.tensor_copy(h_t[:], h0s_t[:])
            nc.vector.tensor_scalar_mul(h0s_t[:], h0s_t[:], ALPHA)
            mlp_scope.__exit__(None, None, None)
            slot_scope = tc.tile_pool(name="slot", bufs=slot_bufs)
            spool = slot_scope.__enter__()

            # ---- propagation steps ----
            SGC = 512
            groups = _make_groups(d_b, SGC)
            # uniform schedule: every block in a group has dmax_g columns
            d_u = np.zeros(NB, dtype=np.int64)
            for (b0, nb, dmax_g) in groups:
                d_u[b0:b0 + nb] = dmax_g
            col_off = np.zeros(NB + 1, dtype=np.int64)
            col_off[1:] = np.cumsum(d_u)

            def step_body(_i):
                import concourse.bass as bass_
                # g = dinv * h
                nc.vector.tensor_tensor(
                    g_t[:], h_t[:],
                    dinv_t[:].rearrange("p (b o) -> p b o", o=1).to_broadcast([P, NB, OUT_CH]),
                    op=mybir.AluOpType.mult)
                nc.gpsimd.dma_start(gshard.ap()[:NRANK, :], g_t[:])
                if do_allgather:
                    nc.gpsimd.collective_compute(
                        "AllGather", mybir.AluOpType.bypass,
                        replica_groups=[list(range(N_CORES))],
                        ins=[gshard.ap()[:, :]],
                        outs=[Gtab.ap()[:, :]],
                    )
                qi = 0
                for (b0, nb, dmax_g) in groups:
                    if not do_gather:
                        break
                    st = spool.tile([P, SGC, OUT_CH], dt, tag="slot")
                    stv = st[:, :nb * dmax_g, :].rearrange(
                        "p (b s) c -> p b s c", b=nb)
                    for j in range(nb):
                        for s in range(dmax_g):
                            col = int(col_off[b0 + j] + s)
                            inst = nc.gpsimd.indirect_dma_start(
                                out=stv[:, j, s, :], out_offset=None, in_=Gtab[:],
                                in_offset=bass_.IndirectOffsetOnAxis(
                                    ap=idx_t[:, col:col + 1], axis=0))
                            q = qi % n_queues
                            qi += 1
                            if q:
                                inst.ins.queue = f"qPoolDynamic{q}"
                    nc.vector.reduce_sum(
                        agg_t[:, b0:b0 + nb, :].rearrange("p b c -> p c b"),
                        stv[:].rearrange("p b s c -> p c b s"),
                        axis=mybir.AxisListType.X)
                # h = 0.9 * dinv * (agg + g) + alpha*h0
                nc.vector.tensor_add(agg_t[:], agg_t[:], g_t[:])
                nc.vector.tensor_tensor(
                    agg_t[:], agg_t[:],
                    dinv_t[:].rearrange("p (b o) -> p b o", o=1).to_broadcast([P, NB, OUT_CH]),
                    op=mybir.AluOpType.mult)
                nc.vector.tensor_scalar_mul(agg_t[:], agg_t[:], 1.0 - ALPHA)
                nc.vector.tensor_add(h_t[:], agg_t[:], h0s_t[:])

            for _step in range(k_steps):
                step_body(_step)

            nc.gpsimd.dma_start(h_out[:], h_t[:])
            slot_scope.__exit__(None, None, None)
    nc.compile()
    return nc


def kernel(x, edge_index, W1, b1, W2, b2):
    per_core, W1p_t, W2a, b2a = _build_host_data(x, edge_index, W1, b1, W2, b2)

    # shared gather schedule: elementwise max of d_b across cores
    d_b = np.max(np.stack([pc["d_b"] for pc in per_core]), axis=0)
    groups = _make_groups(d_b)
    d_u = np.zeros(NB, dtype=np.int64)
    for (b0, nb, dmax_g) in groups:
        d_u[b0:b0 + nb] = dmax_g
    T_g = int(d_u.sum())
    col_off = np.zeros(NB + 1, dtype=np.int64)
    col_off[1:] = np.cumsum(d_u)

    in_maps = []
    for c in range(N_CORES):
        pc = per_core[c]
        idx_pad = np.full((P, T_g), NRANK, dtype=np.int32)  # zero row of core 0
        for b in range(NB):
            db_c = int(pc["d_b"][b])
            if db_c:
                idx_pad[:, col_off[b]:col_off[b] + db_c] = \
                    pc["idx"][:, pc["col_off"][b]:pc["col_off"][b] + db_c]
        in_maps.append({
            "xT": pc["xT"],
            "W1p": W1p_t,
            "W2": W2a,
            "b2": b2a.reshape(OUT_CH, 1),
            "dinv": pc["dinv"],
            "idx": idx_pad,
        })

    nc = _build_bass(d_b, [T_g] * N_CORES)
    global _LAST_NC, _LAST_IN_MAPS
    _LAST_NC, _LAST_IN_MAPS = nc, in_maps
    from concourse import bass_utils
    res = bass_utils.run_bass_kernel_spmd(nc, in_maps, core_ids=list(range(N_CORES)))

    out = np.zeros((N_NODES, OUT_CH), dtype=np.float32)
    for c in range(N_CORES):
        hc = res.results[c]["h_out"].reshape(P, NB, OUT_CH)
        ids_sorted = per_core[c]["ids_sorted"]
        s = np.arange(NS)
        out[ids_sorted] = hc[s % P, s // P, :]
    return out


# revision 13
# speedup vs baseline: 62.0602x; 1.2945x over previous
"""APPNP (MLP + 10-step personalized-pagerank propagation) on 8 trn2 NeuronCores.

Strategy:
- Nodes are dst-sharded across 8 cores (12500 each).
- MLP (x @ W1 -> relu -> @ W2) runs on the tensor engine per core over the
  core's node shard, with x pre-transposed on host (contraction dim on
  partitions) and b1 folded in as an extra ones-row of x.
- Propagation uses the factorized GCN norm: A_hat h = dinv * ((A+I) (dinv*h)),
  so no per-edge norm values are needed: per step each core computes
  g = dinv*h on its shard, AllGathers g into a full table in DRAM, gathers
  g[src] for each in-edge of its shard via indirect DMA into a degree-uniform
  slot layout, reduces slots per dst with one vector-engine reduction per
  128-dst block, adds the self-loop term and the alpha*h0 term.
- Slot layout: per core, dsts sorted by in-degree desc; sorted position
  s <-> (block b = s//128, lane = s%128). Block b's slot count = max degree
  in block (degree-sorted => tiny padding). Pad slots gather a zero row.
"""
import numpy as np

_LAST_NC = None
_LAST_IN_MAPS = None

K = 10
ALPHA = 0.1
N_NODES = 100000
N_CORES = 8
NS = N_NODES // N_CORES          # 12500 dsts per core
NB = 98                           # ceil(12544/128) blocks (12544 = 128*98)
NRANK = 128 * NB                  # 12544 padded ranks per core
SHARD_ROWS = NRANK + 1            # +1 zero row for pad gathers
IN_CH, HID_CH, OUT_CH = 500, 64, 16
KIN = 512                         # padded in_ch (500 feats + 1 bias + pad)
P = 128



def _make_groups(d_b, sgc=1):
    groups = []   # (b0, nb, dmax_g)
    b = 0
    while b < NB:
        dmax_g = max(int(d_b[b]), 1)
        nb = 1
        while (b + nb < NB and (nb + 1) * dmax_g <= sgc
               and int(d_b[b + nb]) <= dmax_g):
            nb += 1
        groups.append((b, nb, dmax_g))
        b += nb
    return groups


def _build_host_data(x, edge_index, W1, b1, W2, b2):
    x = np.asarray(x, dtype=np.float32)
    ei = np.asarray(edge_index)
    src = ei[0].astype(np.int64)
    dst = ei[1].astype(np.int64)

    deg = np.bincount(dst, minlength=N_NODES).astype(np.float32) + 1.0
    dinv = 1.0 / np.sqrt(deg)

    # per-core degree sort of the core's dst shard; global row map for g table
    row_of_node = np.empty(N_NODES, dtype=np.int64)
    perm_per_core = []          # natural ids in sorted order per core
    for c in range(N_CORES):
        ids = np.arange(c * NS, (c + 1) * NS)
        order = np.argsort(-deg[ids], kind="stable")
        ids_sorted = ids[order]
        perm_per_core.append(ids_sorted)
        s = np.arange(NS)
        lane = s % P
        b = s // P
        row_of_node[ids_sorted] = c * SHARD_ROWS + lane * NB + b
    zero_row_of_core0 = NRANK  # row index (within shard) that stays zero

    # per-core slot tables
    per_core = []
    dst_core = dst // NS
    for c in range(N_CORES):
        m = dst_core == c
        src_c = src[m]
        dst_c = dst[m]
        ids_sorted = perm_per_core[c]
        # sorted position of each dst in this core
        pos_of = np.empty(NS, dtype=np.int64)
        pos_of[ids_sorted - c * NS] = np.arange(NS)
        pos = pos_of[dst_c - c * NS]
        lane = pos % P
        blk = pos // P
        degs = deg[ids_sorted].astype(np.int64) - 1   # in-edges only
        d_b = np.zeros(NB, dtype=np.int64)
        for b in range(NB):
            seg = degs[b * P:(b + 1) * P]
            d_b[b] = seg.max() if len(seg) else 0
        d_b = np.maximum(d_b, 0)
        col_off = np.zeros(NB + 1, dtype=np.int64)
        col_off[1:] = np.cumsum(d_b)
        T_g = int(col_off[-1])

        # slot fill: order edges by (blk, lane) then slot rank within dst
        idx_tab = np.full((P, T_g), zero_row_of_core0, dtype=np.int64)
        order2 = np.lexsort((src_c, pos))   # group by dst pos
        pos_s = pos[order2]
        src_s = src_c[order2]
        # rank within each dst
        counts = np.bincount(pos_s, minlength=NS)
        rank = np.arange(len(pos_s)) - np.repeat(
            np.concatenate(([0], np.cumsum(counts)))[:-1], counts)
        lane_s = pos_s % P
        blk_s = pos_s // P
        cols = col_off[blk_s] + rank
        idx_tab[lane_s, cols] = row_of_node[src_s]
        per_core.append(dict(idx=idx_tab.astype(np.int32), d_b=d_b,
                             col_off=col_off, T_g=T_g,
                             ids_sorted=ids_sorted))

    # MLP host prep per core: xT [128, 4, NRANK] fp32, column order = sorted pos
    W1p = np.zeros((KIN, HID_CH), dtype=np.float32)
    W1p[:IN_CH] = np.asarray(W1, dtype=np.float32)
    W1p[IN_CH] = np.asarray(b1, dtype=np.float32)
    W1p_t = W1p.reshape(4, P, HID_CH).transpose(1, 0, 2).copy()  # [128,4,64]
    for c in range(N_CORES):
        ids_sorted = per_core[c]["ids_sorted"]
        xp = np.zeros((KIN, NRANK), dtype=np.float32)
        xp[:IN_CH, :NS] = x[ids_sorted].T
        xp[IN_CH, :NS] = 1.0
        per_core[c]["xT"] = xp.reshape(4, P, NRANK).transpose(1, 0, 2).copy()
        dv = np.zeros((P, NB), dtype=np.float32)
        s = np.arange(NS)
        dv[s % P, s // P] = dinv[ids_sorted]
        per_core[c]["dinv"] = dv
    return per_core, W1p_t, np.asarray(W2, np.float32), np.asarray(b2, np.float32)


def _build_bass(d_b_list, T_g_list, n_queues=4, k_steps=K, do_gather=True, do_allgather=True, slot_bufs=3, sgc=1):
    import concourse.bacc as bacc
    import concourse.mybir as mybir
    import concourse.tile as tile
    import concourse.bass as bass

    # all cores share one program; use the max structure and per-core idx data.
    # d_b differs per core -> use per-column gather driven by a SHARED column
    # count T_max, with per-core idx tables padded to T_max (pad cols gather
    # the zero row into a scratch slot tile and reduce into a junk agg block).
    # Simpler: use the same d_b schedule for all cores = elementwise max over
    # cores (computed on host, passed in d_b_list as the shared schedule).
    d_b = d_b_list
    _groups = _make_groups(d_b, sgc)
    T_g = int(sum(nb * dm for (_b0, nb, dm) in _groups))
    DMAX = int(max(d_b)) if len(d_b) else 1

    nc = bacc.Bacc(None, num_devices=N_CORES, num_swdge_queues=n_queues,
                   dynamic_dma_scratch_size=65536)
    xT = nc.dram_tensor("xT", [P, 4, NRANK], mybir.dt.float32, kind="ExternalInput")
    W1p = nc.dram_tensor("W1p", [P, 4, HID_CH], mybir.dt.float32, kind="ExternalInput")
    W2 = nc.dram_tensor("W2", [HID_CH, OUT_CH], mybir.dt.float32, kind="ExternalInput")
    b2 = nc.dram_tensor("b2", [OUT_CH, 1], mybir.dt.float32, kind="ExternalInput")
    dinv_in = nc.dram_tensor("dinv", [P, NB], mybir.dt.float32, kind="ExternalInput")
    idx_in = nc.dram_tensor("idx", [P, max(T_g, 1)], mybir.dt.int32, kind="ExternalInput")
    h_out = nc.dram_tensor("h_out", [P, NB * OUT_CH], mybir.dt.float32, kind="ExternalOutput")

    gshard = nc.dram_tensor("gshard", [SHARD_ROWS, OUT_CH], mybir.dt.float32)
    Gtab = nc.dram_tensor("Gtab", [SHARD_ROWS * N_CORES, OUT_CH], mybir.dt.float32)

    dt = mybir.dt.float32
    with tile.TileContext(nc) as tc:
        with tc.tile_pool(name="persist", bufs=1) as pers, \
             tc.tile_pool(name="ps", bufs=2, space="PSUM") as pp, \
             tc.tile_pool(name="pst", bufs=2, space="PSUM") as ppt:

            # persistent tiles
            idx_t = pers.tile([P, max(T_g, 1)], mybir.dt.int32)
            nc.gpsimd.dma_start(idx_t[:], idx_in[:])
            dinv_t = pers.tile([P, NB], dt)
            nc.gpsimd.dma_start(dinv_t[:], dinv_in[:])
            w1_t = pers.tile([P, 4, HID_CH], dt)
            nc.gpsimd.dma_start(w1_t[:], W1p[:])
            w2_t = pers.tile([HID_CH, OUT_CH], dt)
            nc.gpsimd.dma_start(w2_t[:], W2[:])
            b2_t = pers.tile([OUT_CH, 1], dt)
            nc.gpsimd.dma_start(b2_t[:], b2[:])
            ident = pers.tile([P, P], dt)
            from concourse.masks import make_identity
            make_identity(nc, ident[:])

            h0s_t = pers.tile([P, NB, OUT_CH], dt)   # alpha * h0
            h_t = pers.tile([P, NB, OUT_CH], dt)     # current h
            g_t = pers.tile([P, NB, OUT_CH], dt)     # dinv * h
            agg_t = pers.tile([P, NB, OUT_CH], dt)
            zrow = pers.tile([1, OUT_CH], dt)
            nc.gpsimd.memset(zrow[:], 0.0)
            nc.gpsimd.dma_start(gshard.ap()[NRANK:NRANK + 1, :], zrow[:])

            # ---- MLP ----
            mlp_scope = tc.tile_pool(name="mlp", bufs=3)
            mpool = mlp_scope.__enter__()
            tiles = [(t * KIN, KIN) for t in range(NRANK // KIN)]
            rem = NRANK - (NRANK // KIN) * KIN
            if rem:
                tiles.append(((NRANK // KIN) * KIN, rem))
            for (c0, w) in tiles:
                xt = mpool.tile([P, 4, KIN], dt, tag="xt")
                nc.gpsimd.dma_start(xt[:, :, :w], xT[:, :, c0:c0 + w])
                ps1 = pp.tile([HID_CH, KIN], dt, tag="ps1")
                for k in range(4):
                    nc.tensor.matmul(ps1[:, :w], w1_t[:, k, :], xt[:, k, :w],
                                     start=(k == 0), stop=(k == 3))
                h1 = mpool.tile([HID_CH, KIN], dt, tag="h1")
                nc.vector.tensor_scalar_max(h1[:, :w], ps1[:, :w], 0.0)
                ps2 = pp.tile([OUT_CH, KIN], dt, tag="ps2")
                nc.tensor.matmul(ps2[:, :w], w2_t[:], h1[:, :w],
                                 start=True, stop=True)
                hT = mpool.tile([OUT_CH, KIN], dt, tag="hT")
                nc.vector.tensor_tensor(hT[:, :w], ps2[:, :w],
                                        b2_t[:].to_broadcast([OUT_CH, w]),
                                        op=mybir.AluOpType.add)
                for j in range(w // P):
                    b = (c0 + j * P) // P
                    pst = ppt.tile([P, OUT_CH], dt, tag="pst")
                    nc.tensor.transpose(pst[:], hT[:, j * P:(j + 1) * P],
                                        ident[:OUT_CH, :OUT_CH])
                    nc.vector.tensor_copy(h0s_t[:, b, :], pst[:])
            # h = h0 ; h0s = alpha*h0
            nc.vector.tensor_copy(h_t[:], h0s_t[:])
            nc.vector.tensor_scalar_mul(h0s_t[:], h0s_t[:], ALPHA)
            mlp_scope.__exit__(None, None, None)
            slot_scope = tc.tile_pool(name="slot", bufs=slot_bufs)
            spool = slot_scope.__enter__()

            # ---- propagation steps ----
            SGC = max(sgc, DMAX)
            groups = _make_groups(d_b, sgc)
            # uniform schedule: every block in a group has dmax_g columns
            d_u = np.zeros(NB, dtype=np.int64)
            for (b0, nb, dmax_g) in groups:
                d_u[b0:b0 + nb] = dmax_g
            col_off = np.zeros(NB + 1, dtype=np.int64)
            col_off[1:] = np.cumsum(d_u)

            def step_body(_i):
                import concourse.bass as bass_
                # g = dinv * h
                nc.vector.tensor_tensor(
                    g_t[:], h_t[:],
                    dinv_t[:].rearrange("p (b o) -> p b o", o=1).to_broadcast([P, NB, OUT_CH]),
                    op=mybir.AluOpType.mult)
                nc.gpsimd.dma_start(gshard.ap()[:NRANK, :], g_t[:])
                if do_allgather:
                    nc.gpsimd.collective_compute(
                        "AllGather", mybir.AluOpType.bypass,
                        replica_groups=[list(range(N_CORES))],
                        ins=[gshard.ap()[:, :]],
                        outs=[Gtab.ap()[:, :]],
                    )
                qi = 0
                for (b0, nb, dmax_g) in groups:
                    if not do_gather:
                        break
                    st = spool.tile([P, SGC, OUT_CH], dt, tag="slot")
                    stv = st[:, :nb * dmax_g, :].rearrange(
                        "p (b s) c -> p b s c", b=nb)
                    for j in range(nb):
                        for s in range(dmax_g):
                            col = int(col_off[b0 + j] + s)
                            inst = nc.gpsimd.indirect_dma_start(
                                out=stv[:, j, s, :], out_offset=None, in_=Gtab[:],
                                in_offset=bass_.IndirectOffsetOnAxis(
                                    ap=idx_t[:, col:col + 1], axis=0))
                            q = qi % n_queues
                            qi += 1
                            if q:
                                inst.ins.queue = f"qPoolDynamic{q}"
                    nc.vector.reduce_sum(
                        agg_t[:, b0:b0 + nb, :].rearrange("p b c -> p c b"),
                        stv[:].rearrange("p b s c -> p c b s"),
                        axis=mybir.AxisListType.X)
                # h = 0.9 * dinv * (agg + g) + alpha*h0
                nc.vector.tensor_add(agg_t[:], agg_t[:], g_t[:])
                nc.vector.tensor_tensor(
                    agg_t[:], agg_t[:],
                    dinv_t[:].rearrange("p (b o) -> p b o", o=1).to_broadcast([P, NB, OUT_CH]),
                    op=mybir.AluOpType.mult)
                nc.vector.tensor_scalar_mul(agg_t[:], agg_t[:], 1.0 - ALPHA)
                nc.vector.tensor_add(h_t[:], agg_t[:], h0s_t[:])

            for _step in range(k_steps):
                step_body(_step)

            nc.gpsimd.dma_start(h_out[:], h_t[:])
            slot_scope.__exit__(None, None, None)
    nc.compile()
    return nc


def kernel(x, edge_index, W1, b1, W2, b2):
    per_core, W1p_t, W2a, b2a = _build_host_data(x, edge_index, W1, b1, W2, b2)

    # shared gather schedule: elementwise max of d_b across cores
    d_b = np.max(np.stack([pc["d_b"] for pc in per_core]), axis=0)
    groups = _make_groups(d_b)
    d_u = np.zeros(NB, dtype=np.int64)
    for (b0, nb, dmax_g) in groups:
        d_u[b0:b0 + nb] = dmax_g
    T_g = int(d_u.sum())
    col_off = np.zeros(NB + 1, dtype=np.int64)
    col_off[1:] = np.cumsum(d_u)

    in_maps = []
    for c in range(N_CORES):
        pc = per_core[c]
        idx_pad = np.full((P, T_g), NRANK, dtype=np.int32)  # zero row of core 0
        for b in range(NB):
            db_c = int(pc["d_b"][b])
            if db_c:
                idx_pad[:, col_off[b]:col_off[b] + db_c] = \
                    pc["idx"][:, pc["col_off"][b]:pc["col_off"][b] + db_c]
        in_maps.append({
            "xT": pc["xT"],
            "W1p": W1p_t,
            "W2": W2a,
            "b2": b2a.reshape(OUT_CH, 1),
            "dinv": pc["dinv"],
            "idx": idx_pad,
        })

    nc = _build_bass(d_b, [T_g] * N_CORES)
    global _LAST_NC, _LAST_IN_MAPS
    _LAST_NC, _LAST_IN_MAPS = nc, in_maps
    from concourse import bass_utils
    res = bass_utils.run_bass_kernel_spmd(nc, in_maps, core_ids=list(range(N_CORES)))

    out = np.zeros((N_NODES, OUT_CH), dtype=np.float32)
    for c in range(N_CORES):
        hc = res.results[c]["h_out"].reshape(P, NB, OUT_CH)
        ids_sorted = per_core[c]["ids_sorted"]
        s = np.arange(NS)
        out[ids_sorted] = hc[s % P, s // P, :]
    return out


# revision 14
# speedup vs baseline: 63.7393x; 1.0271x over previous
"""APPNP (MLP + 10-step personalized-pagerank propagation) on 8 trn2 NeuronCores.

Strategy:
- Nodes are dst-sharded across 8 cores (12500 each).
- MLP (x @ W1 -> relu -> @ W2) runs on the tensor engine per core over the
  core's node shard, with x pre-transposed on host (contraction dim on
  partitions) and b1 folded in as an extra ones-row of x.
- Propagation uses the factorized GCN norm: A_hat h = dinv * ((A+I) (dinv*h)),
  so no per-edge norm values are needed: per step each core computes
  g = dinv*h on its shard, AllGathers g into a full table in DRAM, gathers
  g[src] for each in-edge of its shard via indirect DMA into a degree-uniform
  slot layout, reduces slots per dst with one vector-engine reduction per
  128-dst block, adds the self-loop term and the alpha*h0 term.
- Slot layout: per core, dsts sorted by in-degree desc; sorted position
  s <-> (block b = s//128, lane = s%128). Block b's slot count = max degree
  in block (degree-sorted => tiny padding). Pad slots gather a zero row.
"""
import numpy as np

_LAST_NC = None
_LAST_IN_MAPS = None

K = 10
ALPHA = 0.1
N_NODES = 100000
N_CORES = 8
NS = N_NODES // N_CORES          # 12500 dsts per core
NB = 98                           # ceil(12544/128) blocks (12544 = 128*98)
NRANK = 128 * NB                  # 12544 padded ranks per core
SHARD_ROWS = NRANK + 1            # +1 zero row for pad gathers
IN_CH, HID_CH, OUT_CH = 500, 64, 16
KIN = 512                         # padded in_ch (500 feats + 1 bias + pad)
P = 128



def _make_groups(d_b, sgc=1):
    groups = []   # (b0, nb, dmax_g)
    b = 0
    while b < NB:
        dmax_g = max(int(d_b[b]), 1)
        nb = 1
        while (b + nb < NB and (nb + 1) * dmax_g <= sgc
               and int(d_b[b + nb]) <= dmax_g):
            nb += 1
        groups.append((b, nb, dmax_g))
        b += nb
    return groups


def _build_host_data(x, edge_index, W1, b1, W2, b2):
    x = np.asarray(x, dtype=np.float32)
    ei = np.asarray(edge_index)
    src = ei[0].astype(np.int64)
    dst = ei[1].astype(np.int64)

    deg = np.bincount(dst, minlength=N_NODES).astype(np.float32) + 1.0
    dinv = 1.0 / np.sqrt(deg)

    # per-core degree sort of the core's dst shard; global row map for g table
    row_of_node = np.empty(N_NODES, dtype=np.int64)
    perm_per_core = []          # natural ids in sorted order per core
    for c in range(N_CORES):
        ids = np.arange(c * NS, (c + 1) * NS)
        order = np.argsort(-deg[ids], kind="stable")
        ids_sorted = ids[order]
        perm_per_core.append(ids_sorted)
        s = np.arange(NS)
        lane = s % P
        b = s // P
        row_of_node[ids_sorted] = c * SHARD_ROWS + lane * NB + b
    zero_row_of_core0 = NRANK  # row index (within shard) that stays zero

    # per-core slot tables
    per_core = []
    dst_core = dst // NS
    for c in range(N_CORES):
        m = dst_core == c
        src_c = src[m]
        dst_c = dst[m]
        ids_sorted = perm_per_core[c]
        # sorted position of each dst in this core
        pos_of = np.empty(NS, dtype=np.int64)
        pos_of[ids_sorted - c * NS] = np.arange(NS)
        pos = pos_of[dst_c - c * NS]
        lane = pos % P
        blk = pos // P
        degs = deg[ids_sorted].astype(np.int64) - 1   # in-edges only
        d_b = np.zeros(NB, dtype=np.int64)
        for b in range(NB):
            seg = degs[b * P:(b + 1) * P]
            d_b[b] = seg.max() if len(seg) else 0
        d_b = np.maximum(d_b, 0)
        col_off = np.zeros(NB + 1, dtype=np.int64)
        col_off[1:] = np.cumsum(d_b)
        T_g = int(col_off[-1])

        # slot fill: order edges by (blk, lane) then slot rank within dst
        idx_tab = np.full((P, T_g), zero_row_of_core0, dtype=np.int64)
        order2 = np.lexsort((src_c, pos))   # group by dst pos
        pos_s = pos[order2]
        src_s = src_c[order2]
        # rank within each dst
        counts = np.bincount(pos_s, minlength=NS)
        rank = np.arange(len(pos_s)) - np.repeat(
            np.concatenate(([0], np.cumsum(counts)))[:-1], counts)
        lane_s = pos_s % P
        blk_s = pos_s // P
        cols = col_off[blk_s] + rank
        idx_tab[lane_s, cols] = row_of_node[src_s]
        per_core.append(dict(idx=idx_tab.astype(np.int32), d_b=d_b,
                             col_off=col_off, T_g=T_g,
                             ids_sorted=ids_sorted))

    # MLP host prep per core: xT [128, 4, NRANK] fp32, column order = sorted pos
    W1p = np.zeros((KIN, HID_CH), dtype=np.float32)
    W1p[:IN_CH] = np.asarray(W1, dtype=np.float32)
    W1p[IN_CH] = np.asarray(b1, dtype=np.float32)
    W1p_t = W1p.reshape(4, P, HID_CH).transpose(1, 0, 2).copy()  # [128,4,64]
    for c in range(N_CORES):
        ids_sorted = per_core[c]["ids_sorted"]
        xp = np.zeros((KIN, NRANK), dtype=np.float32)
        xp[:IN_CH, :NS] = x[ids_sorted].T
        xp[IN_CH, :NS] = 1.0
        per_core[c]["xT"] = xp.reshape(4, P, NRANK).transpose(1, 0, 2).copy()
        dv = np.zeros((P, NB), dtype=np.float32)
        s = np.arange(NS)
        dv[s % P, s // P] = dinv[ids_sorted]
        per_core[c]["dinv"] = dv
    return per_core, W1p_t, np.asarray(W2, np.float32), np.asarray(b2, np.float32)


def _build_bass(d_b_list, T_g_list, n_queues=4, k_steps=K, do_gather=True, do_allgather=True, slot_bufs=3, sgc=1):
    import concourse.bacc as bacc
    import concourse.mybir as mybir
    import concourse.tile as tile
    import concourse.bass as bass

    # all cores share one program; use the max structure and per-core idx data.
    # d_b differs per core -> use per-column gather driven by a SHARED column
    # count T_max, with per-core idx tables padded to T_max (pad cols gather
    # the zero row into a scratch slot tile and reduce into a junk agg block).
    # Simpler: use the same d_b schedule for all cores = elementwise max over
    # cores (computed on host, passed in d_b_list as the shared schedule).
    d_b = d_b_list
    _groups = _make_groups(d_b, sgc)
    T_g = int(sum(nb * dm for (_b0, nb, dm) in _groups))
    DMAX = int(max(d_b)) if len(d_b) else 1

    nc = bacc.Bacc(None, num_devices=N_CORES, num_swdge_queues=n_queues,
                   dynamic_dma_scratch_size=131072)
    xT = nc.dram_tensor("xT", [P, 4, NRANK], mybir.dt.float32, kind="ExternalInput")
    W1p = nc.dram_tensor("W1p", [P, 4, HID_CH], mybir.dt.float32, kind="ExternalInput")
    W2 = nc.dram_tensor("W2", [HID_CH, OUT_CH], mybir.dt.float32, kind="ExternalInput")
    b2 = nc.dram_tensor("b2", [OUT_CH, 1], mybir.dt.float32, kind="ExternalInput")
    dinv_in = nc.dram_tensor("dinv", [P, NB], mybir.dt.float32, kind="ExternalInput")
    idx_in = nc.dram_tensor("idx", [P, max(T_g, 1)], mybir.dt.int32, kind="ExternalInput")
    h_out = nc.dram_tensor("h_out", [P, NB * OUT_CH], mybir.dt.float32, kind="ExternalOutput")

    gshard = nc.dram_tensor("gshard", [SHARD_ROWS, OUT_CH], mybir.dt.float32)
    Gtab = nc.dram_tensor("Gtab", [SHARD_ROWS * N_CORES, OUT_CH], mybir.dt.float32)

    dt = mybir.dt.float32
    with tile.TileContext(nc) as tc:
        with tc.tile_pool(name="persist", bufs=1) as pers, \
             tc.tile_pool(name="ps", bufs=2, space="PSUM") as pp, \
             tc.tile_pool(name="pst", bufs=2, space="PSUM") as ppt:

            # persistent tiles
            idx_t = pers.tile([P, max(T_g, 1)], mybir.dt.int32)
            nc.gpsimd.dma_start(idx_t[:], idx_in[:])
            dinv_t = pers.tile([P, NB], dt)
            nc.gpsimd.dma_start(dinv_t[:], dinv_in[:])
            w1_t = pers.tile([P, 4, HID_CH], dt)
            nc.gpsimd.dma_start(w1_t[:], W1p[:])
            w2_t = pers.tile([HID_CH, OUT_CH], dt)
            nc.gpsimd.dma_start(w2_t[:], W2[:])
            b2_t = pers.tile([OUT_CH, 1], dt)
            nc.gpsimd.dma_start(b2_t[:], b2[:])
            ident = pers.tile([P, P], dt)
            from concourse.masks import make_identity
            make_identity(nc, ident[:])

            h0s_t = pers.tile([P, NB, OUT_CH], dt)   # alpha * h0
            h_t = pers.tile([P, NB, OUT_CH], dt)     # current h
            g_t = pers.tile([P, NB, OUT_CH], dt)     # dinv * h
            agg_t = pers.tile([P, NB, OUT_CH], dt)
            zrow = pers.tile([1, OUT_CH], dt)
            nc.gpsimd.memset(zrow[:], 0.0)
            nc.gpsimd.dma_start(gshard.ap()[NRANK:NRANK + 1, :], zrow[:])

            # ---- MLP ----
            mlp_scope = tc.tile_pool(name="mlp", bufs=3)
            mpool = mlp_scope.__enter__()
            tiles = [(t * KIN, KIN) for t in range(NRANK // KIN)]
            rem = NRANK - (NRANK // KIN) * KIN
            if rem:
                tiles.append(((NRANK // KIN) * KIN, rem))
            for (c0, w) in tiles:
                xt = mpool.tile([P, 4, KIN], dt, tag="xt")
                nc.gpsimd.dma_start(xt[:, :, :w], xT[:, :, c0:c0 + w])
                ps1 = pp.tile([HID_CH, KIN], dt, tag="ps1")
                for k in range(4):
                    nc.tensor.matmul(ps1[:, :w], w1_t[:, k, :], xt[:, k, :w],
                                     start=(k == 0), stop=(k == 3))
                h1 = mpool.tile([HID_CH, KIN], dt, tag="h1")
                nc.vector.tensor_scalar_max(h1[:, :w], ps1[:, :w], 0.0)
                ps2 = pp.tile([OUT_CH, KIN], dt, tag="ps2")
                nc.tensor.matmul(ps2[:, :w], w2_t[:], h1[:, :w],
                                 start=True, stop=True)
                hT = mpool.tile([OUT_CH, KIN], dt, tag="hT")
                nc.vector.tensor_tensor(hT[:, :w], ps2[:, :w],
                                        b2_t[:].to_broadcast([OUT_CH, w]),
                                        op=mybir.AluOpType.add)
                for j in range(w // P):
                    b = (c0 + j * P) // P
                    pst = ppt.tile([P, OUT_CH], dt, tag="pst")
                    nc.tensor.transpose(pst[:], hT[:, j * P:(j + 1) * P],
                                        ident[:OUT_CH, :OUT_CH])
                    nc.vector.tensor_copy(h0s_t[:, b, :], pst[:])
            # h = h0 ; h0s = alpha*h0
            nc.vector.tensor_copy(h_t[:], h0s_t[:])
            nc.vector.tensor_scalar_mul(h0s_t[:], h0s_t[:], ALPHA)
            mlp_scope.__exit__(None, None, None)
            slot_scope = tc.tile_pool(name="slot", bufs=slot_bufs)
            spool = slot_scope.__enter__()

            # ---- propagation steps ----
            SGC = max(sgc, DMAX)
            groups = _make_groups(d_b, sgc)
            # uniform schedule: every block in a group has dmax_g columns
            d_u = np.zeros(NB, dtype=np.int64)
            for (b0, nb, dmax_g) in groups:
                d_u[b0:b0 + nb] = dmax_g
            col_off = np.zeros(NB + 1, dtype=np.int64)
            col_off[1:] = np.cumsum(d_u)

            def step_body(_i):
                import concourse.bass as bass_
                # g = dinv * h
                nc.vector.tensor_tensor(
                    g_t[:], h_t[:],
                    dinv_t[:].rearrange("p (b o) -> p b o", o=1).to_broadcast([P, NB, OUT_CH]),
                    op=mybir.AluOpType.mult)
                nc.gpsimd.dma_start(gshard.ap()[:NRANK, :], g_t[:])
                if do_allgather:
                    nc.gpsimd.collective_compute(
                        "AllGather", mybir.AluOpType.bypass,
                        replica_groups=[list(range(N_CORES))],
                        ins=[gshard.ap()[:, :]],
                        outs=[Gtab.ap()[:, :]],
                    )
                qi = 0
                for (b0, nb, dmax_g) in groups:
                    if not do_gather:
                        break
                    st = spool.tile([P, SGC, OUT_CH], dt, tag="slot")
                    stv = st[:, :nb * dmax_g, :].rearrange(
                        "p (b s) c -> p b s c", b=nb)
                    for j in range(nb):
                        for s in range(dmax_g):
                            col = int(col_off[b0 + j] + s)
                            inst = nc.gpsimd.indirect_dma_start(
                                out=stv[:, j, s, :], out_offset=None, in_=Gtab[:],
                                in_offset=bass_.IndirectOffsetOnAxis(
                                    ap=idx_t[:, col:col + 1], axis=0))
                            q = qi % n_queues
                            qi += 1
                            if q:
                                inst.ins.queue = f"qPoolDynamic{q}"
                    nc.vector.reduce_sum(
                        agg_t[:, b0:b0 + nb, :].rearrange("p b c -> p c b"),
                        stv[:].rearrange("p b s c -> p c b s"),
                        axis=mybir.AxisListType.X)
                # h = 0.9 * dinv * (agg + g) + alpha*h0
                nc.vector.tensor_add(agg_t[:], agg_t[:], g_t[:])
                nc.vector.tensor_tensor(
                    agg_t[:], agg_t[:],
                    dinv_t[:].rearrange("p (b o) -> p b o", o=1).to_broadcast([P, NB, OUT_CH]),
                    op=mybir.AluOpType.mult)
                nc.vector.tensor_scalar_mul(agg_t[:], agg_t[:], 1.0 - ALPHA)
                nc.vector.tensor_add(h_t[:], agg_t[:], h0s_t[:])

            for _step in range(k_steps):
                step_body(_step)

            nc.gpsimd.dma_start(h_out[:], h_t[:])
            slot_scope.__exit__(None, None, None)
    nc.compile()
    return nc


def kernel(x, edge_index, W1, b1, W2, b2):
    per_core, W1p_t, W2a, b2a = _build_host_data(x, edge_index, W1, b1, W2, b2)

    # shared gather schedule: elementwise max of d_b across cores
    d_b = np.max(np.stack([pc["d_b"] for pc in per_core]), axis=0)
    groups = _make_groups(d_b)
    d_u = np.zeros(NB, dtype=np.int64)
    for (b0, nb, dmax_g) in groups:
        d_u[b0:b0 + nb] = dmax_g
    T_g = int(d_u.sum())
    col_off = np.zeros(NB + 1, dtype=np.int64)
    col_off[1:] = np.cumsum(d_u)

    in_maps = []
    for c in range(N_CORES):
        pc = per_core[c]
        idx_pad = np.full((P, T_g), NRANK, dtype=np.int32)  # zero row of core 0
        for b in range(NB):
            db_c = int(pc["d_b"][b])
            if db_c:
                idx_pad[:, col_off[b]:col_off[b] + db_c] = \
                    pc["idx"][:, pc["col_off"][b]:pc["col_off"][b] + db_c]
        in_maps.append({
            "xT": pc["xT"],
            "W1p": W1p_t,
            "W2": W2a,
            "b2": b2a.reshape(OUT_CH, 1),
            "dinv": pc["dinv"],
            "idx": idx_pad,
        })

    nc = _build_bass(d_b, [T_g] * N_CORES)
    global _LAST_NC, _LAST_IN_MAPS
    _LAST_NC, _LAST_IN_MAPS = nc, in_maps
    from concourse import bass_utils
    res = bass_utils.run_bass_kernel_spmd(nc, in_maps, core_ids=list(range(N_CORES)))

    out = np.zeros((N_NODES, OUT_CH), dtype=np.float32)
    for c in range(N_CORES):
        hc = res.results[c]["h_out"].reshape(P, NB, OUT_CH)
        ids_sorted = per_core[c]["ids_sorted"]
        s = np.arange(NS)
        out[ids_sorted] = hc[s % P, s // P, :]
    return out
